# revision 1
# baseline (speedup 1.0000x reference)
"""DendriticFullyConnected Trainium2 kernel.

Math (per reference):
  x_c  = x[:, :409];  x_nc = x[:, 409:]
  state = sigmoid(x_nc @ W_non.T + b_non) - 1
  cluster = (x_c * coeff) @ W_nmda.T          # coeff = [1,2,...,2,1]
  pre = cluster + state
  out = pre^2 / (0.25 + pre^2)

Strategy: data-parallel over batch on 8 cores (1024 rows each), weights
replicated.  Host folds coeff into W_nmda, folds b_non in as an extra
contraction row (paired with a ones-row in x), transposes everything so the
contraction dim lands on SBUF partitions, and zero-pads K to multiples of 128:
   k-tiles 0..3   : nmda part (409 -> 512)
   k-tiles 4..32  : non part + bias row (3688 -> 3712)
Device computes outT[o, b] = sum_k wt[k, o] * xt[k, b] with W-stationary
matmuls (lhsT = wt tile [128k, 128o], rhs = xt [128k, 512b], float32r so the
PE runs at 1 cycle/row), two PSUM accumulation groups (nmda / non), then the
sigmoid + Hill epilogue on ACT/DVE.  Output is [O, B_loc]; host transposes
back and concatenates.

Scheduling: the x shard (16.5 MB) is cached in SBUF but its fill is
HBM-bound (~50 us) while the PE only has ~14 us of work per cached output
tile — a naive o-tile loop stalls ~30 us waiting for the tail of the fill.
So the first 4 o-tiles run k-OUTER: their nmda phases first (need only x
k-tiles 0..3), then all four non-phases advance one k-group at a time
(8 matmuls = ~1.8 us of PE work per arriving 1.5 us x tile), using all
8 PSUM banks.  Remaining o-tiles run the plain o-outer loop.

DMA layout: W streams through Sync/HWDGE in 4-k-tile chunks (the latency
critical feed for the PE); the one-time x cache fill and the output stores go
through GpSimd queues so they never head-of-line-block the W stream.
"""

import numpy as np

B = 8192
IN_F = 4096
OUT_F = 4096
IC = 409                      # clustering synapses
INC = IN_F - IC               # 3687
KD = 0.25                     # Hill k_d = k_a^n = 0.5^2
NCORES = 8
BLOC = B // NCORES            # 1024
KNM_PAD = 512                 # nmda contraction, padded
KNM_TILES = KNM_PAD // 128    # 4
KNON_PAD = 3712               # non contraction + bias row (3688), padded
KS = (KNM_PAD + KNON_PAD) // 128   # 33
KPAD = KS * 128               # 4224
BIAS_ROW = KNM_PAD + INC      # 4199: ones-row in x / b_non row in wt
OT = OUT_F // 128             # 32 output-row tiles
NBH = BLOC // 512             # 2 batch halves (512 = max fp32 matmul free dim)
OT_AHEAD = 4                  # o-tiles run k-outer to cover the x-cache fill

# Non-phase W-chunk schedule: groups of k-tiles fetched in one DMA each (7x4+1).
NON_GROUPS = [(k, min(4, KS - k)) for k in range(KNM_TILES, KS, 4)]

_nc_cache = []


def _build():
    import concourse.bacc as bacc
    import concourse.tile as tile
    import concourse.mybir as mybir

    f32 = mybir.dt.float32
    f32r = mybir.dt.float32r
    ACT = mybir.ActivationFunctionType

    nc = bacc.Bacc(None, target_bir_lowering=False)
    xt = nc.dram_tensor("xt", [KPAD, BLOC], f32, kind="ExternalInput")
    wt = nc.dram_tensor("wt", [KPAD, OUT_F], f32, kind="ExternalInput")
    outT = nc.dram_tensor("outT", [OUT_F, BLOC], f32, kind="ExternalOutput")

    with tile.TileContext(nc) as tc:
        with (
            tc.tile_pool(name="xpool", bufs=1) as xpool,
            tc.tile_pool(name="wpool", bufs=8) as wpool,
            tc.tile_pool(name="nmpool", bufs=12) as nmpool,
            tc.tile_pool(name="tmp", bufs=8) as tmp,
            tc.tile_pool(name="psum", bufs=8, space="PSUM") as psum,
        ):
            # Cache the full x shard in SBUF: 33 k-tiles of [128, 1024] f32r.
            # GpSimd queues: keeps the W stream on Sync unblocked.
            # Rows 409..511 and 4200..4223 of xT are structural zero padding:
            # memset them instead of spending fill-window HBM bandwidth.
            PAD = {3: 409 - 3 * 128, 32: BIAS_ROW + 1 - 32 * 128}  # real rows
            # The x fill is split across both DMA paths (even k-tiles on
            # Sync/HWDGE, odd on GpSimd/SWDGE) — a single path's queue set
            # caps well below what the HBM can deliver.  The Sync-side DMAs
            # are emitted lazily (interleaved with the W-chunk stream) via
            # feed_x() so they don't head-of-line-block W issues.
            xk = []
            x_pending = []
            for ks in range(KS):
                t = xpool.tile([128, BLOC], f32r, tag=f"x{ks}")
                rows = PAD.get(ks, 128)
                if rows < 128:
                    nc.vector.memset(t[:].bitcast(f32), 0.0)
                src = xt[ks * 128 : ks * 128 + rows, :].bitcast(f32r)
                if ks % 2 == 1 or ks < 4:
                    nc.gpsimd.dma_start(t[:rows, :], src)
                else:
                    x_pending.append((t, rows, src))
                xk.append(t)
            x_pending.reverse()  # pop() from the front of the schedule

            def feed_x(n):
                for _ in range(n):
                    if x_pending:
                        t, rows, src = x_pending.pop()
                        nc.sync.dma_start(t[:rows, :], src)

            def osl(ot):
                return slice(ot * 128, (ot + 1) * 128)

            def load_w_group(k0, g, ot):
                wg = wpool.tile([128, 4, 128], f32r, tag="w", name=f"w_{ot}_{k0}")
                src = wt[k0 * 128 : (k0 + g) * 128, osl(ot)].bitcast(f32r)
                nc.sync.dma_start(
                    wg[:, :g, :], src.rearrange("(g p) o -> p g o", p=128)
                )
                return wg

            def mm_sweep(psl, wg, k0, g, first_k, last_k):
                for j in range(g):
                    ks = k0 + j
                    for bh in range(NBH):
                        nc.tensor.matmul(
                            psl[bh][:],
                            lhsT=wg[:, j, :],
                            rhs=xk[ks][:, bh * 512 : (bh + 1) * 512],
                            start=(ks == first_k),
                            stop=(ks == last_k),
                        )

            def nmda_phase(ot):
                psn = [
                    psum.tile([128, 512], f32, tag="ps", name=f"psn_{ot}_{i}")
                    for i in range(NBH)
                ]
                wg = load_w_group(0, KNM_TILES, ot)
                feed_x(1)
                mm_sweep(psn, wg, 0, KNM_TILES, 0, KNM_TILES - 1)
                nm = []
                for bh in range(NBH):
                    t = nmpool.tile([128, 512], f32, tag="nm", name=f"nm_{ot}_{bh}")
                    nc.scalar.copy(t[:], psn[bh][:])
                    nm.append(t)
                return nm

            def epilogue_pair(ot, ps_pair, nm_pair):
                # pre = nm - sigmoid(-(z+b));  out = pre^2 / (KD + pre^2)
                # The two batch-half chains are interleaved so ACT and DVE
                # overlap instead of running one serial chain after the other.
                sig = [
                    tmp.tile([128, 512], f32, tag="t", name=f"sig_{ot}_{bh}")
                    for bh in range(NBH)
                ]
                rec = [
                    tmp.tile([128, 512], f32, tag="t", name=f"rec_{ot}_{bh}")
                    for bh in range(NBH)
                ]
                for bh in range(NBH):
                    nc.scalar.activation(sig[bh][:], ps_pair[bh][:], ACT.Sigmoid, scale=-1.0)
                for bh in range(NBH):
                    nc.vector.tensor_sub(sig[bh][:], nm_pair[bh][:], sig[bh][:])  # := pre
                for bh in range(NBH):
                    nc.scalar.activation(nm_pair[bh][:], sig[bh][:], ACT.Square)  # := pre^2
                for bh in range(NBH):
                    nc.vector.tensor_scalar_add(sig[bh][:], nm_pair[bh][:], KD)   # := den
                for bh in range(NBH):
                    nc.vector.reciprocal_approx_fast(rec[bh][:], sig[bh][:])
                for bh in range(NBH):
                    nc.vector.tensor_mul(nm_pair[bh][:], nm_pair[bh][:], rec[bh][:])
                for bh in range(NBH):
                    # ACT is the second HWDGE engine: stores ride its FIFO where
                    # they follow the epilogue anyway, never blocking the W
                    # stream on Sync and never paying the slow SWDGE tail drain.
                    bsl = slice(bh * 512, (bh + 1) * 512)
                    nc.scalar.dma_start(outT[osl(ot), bsl], nm_pair[bh][:])

            # ── Phase A: nmda for the first OT_AHEAD o-tiles (needs xk[0..3]) ──
            nm_ahead = [nmda_phase(ot) for ot in range(OT_AHEAD)]

            # ── Phase B: k-outer non-accumulation across those o-tiles ──
            ps_ahead = [
                [
                    psum.tile([128, 512], f32, tag="ps", name=f"psB_{ot}_{i}")
                    for i in range(NBH)
                ]
                for ot in range(OT_AHEAD)
            ]
            for k0, g in NON_GROUPS:
                wgs = [load_w_group(k0, g, ot) for ot in range(OT_AHEAD)]
                feed_x(2)
                # j-outer: each arriving x k-tile unlocks 2*OT_AHEAD matmuls,
                # keeping PE gaps below the HAM re-throttle window during the
                # x-cache fill.
                for j in range(g):
                    ks = k0 + j
                    for ot in range(OT_AHEAD):
                        for bh in range(NBH):
                            nc.tensor.matmul(
                                ps_ahead[ot][bh][:],
                                lhsT=wgs[ot][:, j, :],
                                rhs=xk[ks][:, bh * 512 : (bh + 1) * 512],
                                start=(ks == KNM_TILES),
                                stop=(ks == KS - 1),
                            )
            for ot in range(OT_AHEAD):
                epilogue_pair(ot, ps_ahead[ot], nm_ahead[ot])

            # ── Phase C: remaining o-tiles, plain o-outer loop ──
            for ot in range(OT_AHEAD, OT):
                nm = nmda_phase(ot)
                ps = [
                    psum.tile([128, 512], f32, tag="ps", name=f"ps_{ot}_{i}")
                    for i in range(NBH)
                ]
                for k0, g in NON_GROUPS:
                    wg = load_w_group(k0, g, ot)
                    feed_x(2)
                    mm_sweep(ps, wg, k0, g, KNM_TILES, KS - 1)
                epilogue_pair(ot, ps, nm)
    nc.compile()
    return nc


def _warmup():
    """Tiny throwaway NEFF run: the first execution after session start
    occasionally dies with NRT_EXEC_UNIT_UNRECOVERABLE; absorb that here."""
    import concourse.bacc as bacc
    import concourse.tile as tile
    import concourse.mybir as mybir
    from concourse.bass_utils import run_bass_kernel_spmd

    nc = bacc.Bacc(None, target_bir_lowering=False)
    a = nc.dram_tensor("a", [128, 128], mybir.dt.float32, kind="ExternalInput")
    b = nc.dram_tensor("b", [128, 128], mybir.dt.float32, kind="ExternalOutput")
    with tile.TileContext(nc) as tc:
        with tc.tile_pool(name="p", bufs=1) as pool:
            t = pool.tile([128, 128], mybir.dt.float32)
            nc.sync.dma_start(t[:], a[:])
            nc.sync.dma_start(b[:], t[:])
    nc.compile()
    ins = [{"a": np.zeros((128, 128), np.float32)} for _ in range(NCORES)]
    for _ in range(3):
        try:
            run_bass_kernel_spmd(nc, ins, core_ids=list(range(NCORES)))
            return
        except Exception:
            continue


def kernel(x, W_nmda, W_non, b_non):
    from concourse.bass_utils import run_bass_kernel_spmd

    x = np.asarray(x, dtype=np.float32)
    W_nmda = np.asarray(W_nmda, dtype=np.float32)
    W_non = np.asarray(W_non, dtype=np.float32)
    b_non = np.asarray(b_non, dtype=np.float32)

    coeff = np.full((IC,), 2.0, dtype=np.float32)
    coeff[0] = 1.0
    coeff[-1] = 1.0

    xT = np.zeros((KPAD, B), dtype=np.float32)
    xT[0:IC] = x[:, :IC].T
    xT[KNM_PAD : KNM_PAD + INC] = x[:, IC:].T
    xT[BIAS_ROW] = 1.0

    wt = np.zeros((KPAD, OUT_F), dtype=np.float32)
    wt[0:IC] = (W_nmda * coeff[None, :]).T
    wt[KNM_PAD : KNM_PAD + INC] = W_non.T
    wt[BIAS_ROW] = b_non

    in_maps = [
        {
            "xt": np.ascontiguousarray(xT[:, c * BLOC : (c + 1) * BLOC]),
            "wt": wt,
        }
        for c in range(NCORES)
    ]

    if not _nc_cache:
        _warmup()
        _nc_cache.append(_build())
    nc = _nc_cache[0]

    res = None
    last_exc = None
    for _attempt in range(3):
        try:
            res = run_bass_kernel_spmd(nc, in_maps, core_ids=list(range(NCORES)))
            break
        except Exception as e:  # transient device errors (e.g. first-run NRT hiccup)
            last_exc = e
    if res is None:
        raise last_exc

    global LAST_RESULT
    LAST_RESULT = res

    out = np.empty((B, OUT_F), dtype=np.float32)
    for c in range(NCORES):
        out[c * BLOC : (c + 1) * BLOC] = res.results[c]["outT"].T
    return out


LAST_RESULT = None



# revision 5
# speedup vs baseline: 1.6700x; 1.6700x over previous
"""DendriticFullyConnected Trainium2 kernel — mixed bf16 / fp8-DoubleRow.

Math (per reference):
  x_c  = x[:, :409];  x_nc = x[:, 409:]
  state = sigmoid(x_nc @ W_non.T + b_non) - 1
  cluster = (x_c * coeff) @ W_nmda.T          # coeff = [1,2,...,2,1]
  pre = cluster + state
  out = pre^2 / (0.25 + pre^2)

Strategy: data-parallel over batch on 8 cores (1024 rows each), weights
replicated.  The contraction splits by precision sensitivity:

  nmda part (K=409->512, 4 k-tiles)  : bf16.  cluster hits the Hill directly
    (sigma~2, gain ~1), so fp8 here costs ~5e-2 rel err.  bf16 keeps it at
    ~3e-3 and runs at the same 1 cycle/row as fp32r, with half the DMA.
  non part (K=3687+bias->3840, 15 pairs of k-tiles): fp8 e4m3 with
    perf_mode=DoubleRow (2 fp8 weights per PE cell -> 256-deep contraction
    per matmul at ~0.55 cycle/row).  The sigmoid's <=0.25 gain squashes the
    fp8 quantization noise (measured 6.3e-3 rel-l2 end to end, vs 2e-2 gate).
    W_non/b_non are pre-scaled by 64 so sigma~1 lands mid e4m3 range (away
    from subnormals); the 1/64 is folded into the sigmoid activation scale.

Layouts are all host-prepared so every DMA is a straight contiguous copy:
  xnm [512, 1024/core] bf16;  xnn [15kp*128p, 2i*1024b] fp8 (i = DoubleRow
  half, logical k = kp*256 + i*128 + p);  wnm rows ot*128+p, cols kt*128+o;
  wnn rows ot*128+p, cols kp*256 + i*128 + o.  Bias rides as x-row 3687
  (ones) paired with b_non*64 in wnn.

Device: outT[o, b] = sum_k wt[k, o] xt[k, b] with W-stationary matmuls
(lhsT = w tile, rhs = cached x), two PSUM groups (nmda / non) per o-tile,
then sigmoid + Hill epilogue on ACT/DVE.  Host transposes back.

Scheduling: the x shard (~5 MB) is cached in SBUF.  Phase A runs the bf16
nmda phases of the first OT_AHEAD o-tiles (they only need the 1 MB xnm)
while the xnn fill streams (xnm + odd kp on GpSimd/SWDGE, even kp lazily
interleaved with the W stream on Sync/HWDGE).  Phases B/C are the plain
o-outer loop; W (622 KB/o-tile) streams on Sync, output stores ride the
ACT HWDGE queue.
"""

import numpy as np
import ml_dtypes

B = 8192
IN_F = 4096
OUT_F = 4096
IC = 409                      # clustering synapses
INC = IN_F - IC               # 3687
KD = 0.25                     # Hill k_d = k_a^n = 0.5^2
NCORES = 8
BLOC = B // NCORES            # 1024
OT = OUT_F // 128             # 32 output-row tiles
NBH = BLOC // 512             # 2 batch halves (512 = max fp32 matmul free dim)
OT_AHEAD = 8                  # o-tiles whose nmda phase covers the x fill

KNM_PAD = 512                 # nmda contraction, padded (4 k-tiles, bf16)
KNM_TILES = 4
KNN = INC + 1                 # 3688: non contraction + bias row
KP = 15                       # fp8 DoubleRow k-pairs (15 * 256 = 3840)
KNN_PAD = KP * 256
S_W = 64.0                    # fp8 pre-scale on W_non/b_non

_nc_cache = []


def _build():
    import concourse.bacc as bacc
    import concourse.tile as tile
    import concourse.mybir as mybir

    f32 = mybir.dt.float32
    bf16 = mybir.dt.bfloat16
    f8 = mybir.dt.float8e4
    ACT = mybir.ActivationFunctionType
    DR = mybir.MatmulPerfMode.DoubleRow

    nc = bacc.Bacc(None, target_bir_lowering=False)
    xnm = nc.dram_tensor("xnm", [KNM_PAD, BLOC], bf16, kind="ExternalInput")
    xnn = nc.dram_tensor("xnn", [KP * 128, 2 * BLOC], f8, kind="ExternalInput")
    wnm = nc.dram_tensor("wnm", [OUT_F, KNM_PAD], bf16, kind="ExternalInput")
    wnn = nc.dram_tensor("wnn", [OUT_F, KP * 256], f8, kind="ExternalInput")
    outT = nc.dram_tensor("outT", [OUT_F, BLOC], f32, kind="ExternalOutput")

    with tile.TileContext(nc) as tc:
        with (
            tc.tile_pool(name="xpool", bufs=1) as xpool,
            tc.tile_pool(name="wmpool", bufs=3) as wmpool,
            tc.tile_pool(name="wnpool", bufs=4) as wnpool,
            tc.tile_pool(name="nmpool", bufs=24) as nmpool,
            tc.tile_pool(name="tmp", bufs=8) as tmp,
            tc.tile_pool(name="psum", bufs=8, space="PSUM") as psum,
        ):
            # ── x cache fill ────────────────────────────────────────────
            # xnm (needed first, by phase A) + odd-kp xnn go on GpSimd;
            # even-kp xnn DMAs are emitted lazily between W issues on Sync
            # so they never head-of-line-block the W stream.
            xm = []
            for kt in range(KNM_TILES):
                t = xpool.tile([128, BLOC], bf16, tag=f"xm{kt}")
                nc.gpsimd.dma_start(t[:], xnm[kt * 128 : (kt + 1) * 128, :])
                xm.append(t)
            xn = []
            x_pending = []
            for kp in range(KP):
                t = xpool.tile([128, 2, BLOC], f8, tag=f"xn{kp}")
                src = xnn[kp * 128 : (kp + 1) * 128, :].rearrange(
                    "p (i b) -> p i b", i=2
                )
                if kp % 2 == 1:
                    nc.gpsimd.dma_start(t[:], src)
                else:
                    x_pending.append((t, src))
                xn.append(t)
            x_pending.reverse()  # pop() from the front of the schedule

            def feed_x(n):
                for _ in range(n):
                    if x_pending:
                        t, src = x_pending.pop()
                        nc.sync.dma_start(t[:], src)

            def osl(ot):
                return slice(ot * 128, (ot + 1) * 128)

            def bsl(bh):
                return slice(bh * 512, (bh + 1) * 512)

            def nmda_phase(ot):
                wg = wmpool.tile([128, KNM_TILES, 128], bf16, tag="wm", name=f"wm_{ot}")
                nc.sync.dma_start(
                    wg[:], wnm[osl(ot), :].rearrange("p (k o) -> p k o", k=KNM_TILES)
                )
                feed_x(1)
                psn = [
                    psum.tile([128, 512], f32, tag="ps", name=f"psn_{ot}_{i}")
                    for i in range(NBH)
                ]
                for kt in range(KNM_TILES):
                    for bh in range(NBH):
                        nc.tensor.matmul(
                            psn[bh][:],
                            lhsT=wg[:, kt, :],
                            rhs=xm[kt][:, bsl(bh)],
                            start=(kt == 0),
                            stop=(kt == KNM_TILES - 1),
                        )
                nm = []
                for bh in range(NBH):
                    t = nmpool.tile([128, 512], f32, tag="nm", name=f"nm_{ot}_{bh}")
                    nc.scalar.copy(t[:], psn[bh][:])
                    nm.append(t)
                return nm

            wn_tiles = {}

            def prefetch_wn(ot):
                if ot not in wn_tiles:
                    wg = wnpool.tile([128, KP, 2, 128], f8, tag="wn", name=f"wn_{ot}")
                    nc.sync.dma_start(
                        wg[:],
                        wnn[osl(ot), :].rearrange("p (k i o) -> p k i o", k=KP, i=2),
                    )
                    feed_x(1)
                    wn_tiles[ot] = wg

            def get_wn(ot):
                prefetch_wn(ot)
                return wn_tiles.pop(ot)

            def non_phase(ot):
                wg = get_wn(ot)
                ps = [
                    psum.tile([128, 512], f32, tag="ps", name=f"ps_{ot}_{i}")
                    for i in range(NBH)
                ]
                for kp in range(KP):
                    for bh in range(NBH):
                        nc.tensor.matmul(
                            ps[bh][:],
                            lhsT=wg[:, kp, :, :],
                            rhs=xn[kp][:, :, bsl(bh)],
                            start=(kp == 0),
                            stop=(kp == KP - 1),
                            perf_mode=DR,
                        )
                return ps

            def epilogue_pair(ot, ps_pair, nm_pair):
                # psum = S_W*(z+b); pre = nm - sigmoid(-(z+b));
                # out = pre^2 / (KD + pre^2).  Chains interleaved so ACT and
                # DVE overlap across the two batch halves.
                sig = [
                    tmp.tile([128, 512], f32, tag="t", name=f"sig_{ot}_{bh}")
                    for bh in range(NBH)
                ]
                rec = [
                    tmp.tile([128, 512], f32, tag="t", name=f"rec_{ot}_{bh}")
                    for bh in range(NBH)
                ]
                for bh in range(NBH):
                    nc.scalar.activation(
                        sig[bh][:], ps_pair[bh][:], ACT.Sigmoid, scale=-1.0 / S_W
                    )
                for bh in range(NBH):
                    nc.vector.tensor_sub(sig[bh][:], nm_pair[bh][:], sig[bh][:])  # pre
                for bh in range(NBH):
                    nc.scalar.activation(nm_pair[bh][:], sig[bh][:], ACT.Square)
                for bh in range(NBH):
                    nc.vector.tensor_scalar_add(sig[bh][:], nm_pair[bh][:], KD)
                for bh in range(NBH):
                    nc.vector.reciprocal_approx_fast(rec[bh][:], sig[bh][:])
                for bh in range(NBH):
                    nc.vector.tensor_mul(nm_pair[bh][:], nm_pair[bh][:], rec[bh][:])
                for bh in range(NBH):
                    # stores ride the ACT HWDGE FIFO: they follow the epilogue
                    # anyway and never block the W stream on Sync.
                    nc.scalar.dma_start(outT[osl(ot), bsl(bh)], nm_pair[bh][:])

            # ── Phase A: nmda for the first OT_AHEAD o-tiles (xnm only) ──
            nm_ahead = []
            for ot in range(OT_AHEAD):
                nm_ahead.append(nmda_phase(ot))
                if ot == 2:
                    prefetch_wn(0)
                elif ot == 5:
                    prefetch_wn(1)

            # ── Phase B: non + epilogue for the ahead o-tiles ──
            for ot in range(OT_AHEAD):
                ps = non_phase(ot)
                epilogue_pair(ot, ps, nm_ahead[ot])

            # ── Phase C: remaining o-tiles, plain o-outer loop ──
            for ot in range(OT_AHEAD, OT):
                nm = nmda_phase(ot)
                ps = non_phase(ot)
                epilogue_pair(ot, ps, nm)
    nc.compile()
    return nc


def _warmup():
    """Tiny throwaway NEFF run: the first execution after session start
    occasionally dies with NRT_EXEC_UNIT_UNRECOVERABLE; absorb that here."""
    import concourse.bacc as bacc
    import concourse.tile as tile
    import concourse.mybir as mybir
    from concourse.bass_utils import run_bass_kernel_spmd

    nc = bacc.Bacc(None, target_bir_lowering=False)
    a = nc.dram_tensor("a", [128, 128], mybir.dt.float32, kind="ExternalInput")
    b = nc.dram_tensor("b", [128, 128], mybir.dt.float32, kind="ExternalOutput")
    with tile.TileContext(nc) as tc:
        with tc.tile_pool(name="p", bufs=1) as pool:
            t = pool.tile([128, 128], mybir.dt.float32)
            nc.sync.dma_start(t[:], a[:])
            nc.sync.dma_start(b[:], t[:])
    nc.compile()
    ins = [{"a": np.zeros((128, 128), np.float32)} for _ in range(NCORES)]
    for _ in range(3):
        try:
            run_bass_kernel_spmd(nc, ins, core_ids=list(range(NCORES)))
            return
        except Exception:
            continue


def kernel(x, W_nmda, W_non, b_non):
    from concourse.bass_utils import run_bass_kernel_spmd

    x = np.asarray(x, dtype=np.float32)
    W_nmda = np.asarray(W_nmda, dtype=np.float32)
    W_non = np.asarray(W_non, dtype=np.float32)
    b_non = np.asarray(b_non, dtype=np.float32)

    coeff = np.full((IC,), 2.0, dtype=np.float32)
    coeff[0] = 1.0
    coeff[-1] = 1.0

    bf16 = ml_dtypes.bfloat16
    f8 = ml_dtypes.float8_e4m3

    # x, nmda part: [512, B] bf16
    xTm = np.zeros((KNM_PAD, B), dtype=np.float32)
    xTm[0:IC] = x[:, :IC].T
    xTm = xTm.astype(bf16)

    # x, non part: logical k = kp*256 + i*128 + p -> [15*128 rows, 2*B] fp8
    xTn = np.zeros((KNN_PAD, B), dtype=np.float32)
    xTn[0:INC] = x[:, IC:].T
    xTn[INC] = 1.0  # bias row
    xTn = (
        xTn.reshape(KP, 2, 128, B).transpose(0, 2, 1, 3).reshape(KP * 128, 2 * B)
    ).astype(f8)

    # W, nmda part: row ot*128+p, col kt*128+o, bf16
    wTm = np.zeros((KNM_PAD, OUT_F), dtype=np.float32)
    wTm[0:IC] = (W_nmda * coeff[None, :]).T
    wnm = (
        wTm.reshape(KNM_TILES, 128, OT, 128)
        .transpose(2, 1, 0, 3)
        .reshape(OUT_F, KNM_PAD)
    ).astype(bf16)

    # W, non part (pre-scaled by S_W): row ot*128+p, col kp*256+i*128+o, fp8
    wTn = np.zeros((KNN_PAD, OUT_F), dtype=np.float32)
    wTn[0:INC] = W_non.T * S_W
    wTn[INC] = b_non * S_W
    wnn = (
        wTn.reshape(KP, 2, 128, OT, 128)
        .transpose(3, 2, 0, 1, 4)
        .reshape(OUT_F, KP * 256)
    ).astype(f8)

    in_maps = [
        {
            "xnm": np.ascontiguousarray(
                xTm.reshape(KNM_PAD, NCORES, BLOC)[:, c, :]
            ),
            "xnn": np.ascontiguousarray(
                xTn.reshape(KP * 128, 2, NCORES, BLOC)[:, :, c, :].reshape(
                    KP * 128, 2 * BLOC
                )
            ),
            "wnm": wnm,
            "wnn": wnn,
        }
        for c in range(NCORES)
    ]

    if not _nc_cache:
        _warmup()
        _nc_cache.append(_build())
    nc = _nc_cache[0]

    res = None
    last_exc = None
    for _attempt in range(3):
        try:
            res = run_bass_kernel_spmd(nc, in_maps, core_ids=list(range(NCORES)))
            break
        except Exception as e:  # transient device errors (e.g. first-run NRT hiccup)
            last_exc = e
    if res is None:
        raise last_exc

    global LAST_RESULT
    LAST_RESULT = res

    out = np.empty((B, OUT_F), dtype=np.float32)
    for c in range(NCORES):
        out[c * BLOC : (c + 1) * BLOC] = res.results[c]["outT"].T
    return out


LAST_RESULT = None


# revision 10
# speedup vs baseline: 1.7914x; 1.0727x over previous
"""DendriticFullyConnected Trainium2 kernel — mixed bf16 / fp8-DoubleRow.

Math (per reference):
  x_c  = x[:, :409];  x_nc = x[:, 409:]
  state = sigmoid(x_nc @ W_non.T + b_non) - 1
  cluster = (x_c * coeff) @ W_nmda.T          # coeff = [1,2,...,2,1]
  pre = cluster + state
  out = pre^2 / (0.25 + pre^2)

Strategy: data-parallel over batch on 8 cores (1024 rows each), weights
replicated.  The contraction splits by precision sensitivity:

  nmda part (K=409->512, 4 k-tiles)  : bf16.  cluster hits the Hill directly
    (sigma~2, gain ~1), so fp8 here costs ~5e-2 rel err.  bf16 keeps it at
    ~3e-3 and runs at the same 1 cycle/row as fp32r, with half the DMA.
  non part (K=3687+bias->3840, 15 pairs of k-tiles): fp8 e4m3 with
    perf_mode=DoubleRow (2 fp8 weights per PE cell -> 256-deep contraction
    per matmul at ~0.55 cycle/row).  The sigmoid's <=0.25 gain squashes the
    fp8 quantization noise (measured 6.3e-3 rel-l2 end to end, vs 2e-2 gate).
    W_non/b_non are pre-scaled by 64 so sigma~1 lands mid e4m3 range (away
    from subnormals); the 1/64 is folded into the sigmoid activation scale.

Layouts are all host-prepared so every DMA is a straight contiguous copy:
  xnm [512, 1024/core] bf16;  xnn [15kp*128p, 2i*1024b] fp8 (i = DoubleRow
  half, logical k = kp*256 + i*128 + p);  wnm rows ot*128+p, cols kt*128+o;
  wnn rows ot*128+p, cols kp*256 + i*128 + o.  Bias rides as x-row 3687
  (ones) paired with b_non*64 in wnn.

Device: outT[o, b] = sum_k wt[k, o] xt[k, b] with W-stationary matmuls
(lhsT = w tile, rhs = cached x), two PSUM groups (nmda / non) per o-tile,
then sigmoid + Hill epilogue on ACT/DVE.  Host transposes back.

Scheduling: the x shard (~5 MB) is cached in SBUF.  Phase A runs the bf16
nmda phases of the first OT_AHEAD o-tiles (they only need the 1 MB xnm)
while the xnn fill streams (xnm + odd kp on GpSimd/SWDGE, even kp lazily
interleaved with the W stream on Sync/HWDGE).  Phases B/C are the plain
o-outer loop; W (622 KB/o-tile) streams on Sync, output stores ride the
ACT HWDGE queue.
"""

import numpy as np
import ml_dtypes

B = 8192
IN_F = 4096
OUT_F = 4096
IC = 409                      # clustering synapses
INC = IN_F - IC               # 3687
KD = 0.25                     # Hill k_d = k_a^n = 0.5^2
NCORES = 8
BLOC = B // NCORES            # 1024
OT = OUT_F // 128             # 32 output-row tiles
NBH = BLOC // 512             # 2 batch halves (512 = max fp32 matmul free dim)
OT_AHEAD = 10                 # o-tiles whose nmda phase covers the x fill

KNM_PAD = 512                 # nmda contraction, padded (4 k-tiles, bf16)
KNM_TILES = 4
KNN = INC + 1                 # 3688: non contraction + bias row
KP = 15                       # fp8 DoubleRow k-pairs (15 * 256 = 3840)
KNN_PAD = KP * 256
S_W = 64.0                    # fp8 pre-scale on W_non/b_non

_nc_cache = []


def _build():
    import concourse.bacc as bacc
    import concourse.tile as tile
    import concourse.mybir as mybir

    f32 = mybir.dt.float32
    bf16 = mybir.dt.bfloat16
    f8 = mybir.dt.float8e4
    ACT = mybir.ActivationFunctionType
    DR = mybir.MatmulPerfMode.DoubleRow

    nc = bacc.Bacc(None, target_bir_lowering=False)
    xnm = nc.dram_tensor("xnm", [KNM_PAD, BLOC], bf16, kind="ExternalInput")
    xnn = nc.dram_tensor("xnn", [KP * 128, 2 * BLOC], f8, kind="ExternalInput")
    wnm = nc.dram_tensor("wnm", [OUT_F, KNM_PAD], bf16, kind="ExternalInput")
    wnn = nc.dram_tensor("wnn", [OUT_F, KP * 256], f8, kind="ExternalInput")
    outT = nc.dram_tensor("outT", [OUT_F, BLOC], bf16, kind="ExternalOutput")

    with tile.TileContext(nc) as tc:
        with (
            tc.tile_pool(name="xpool", bufs=1) as xpool,
            tc.tile_pool(name="wmpool", bufs=3) as wmpool,
            tc.tile_pool(name="wnpool", bufs=4) as wnpool,
            tc.tile_pool(name="nmpool", bufs=24) as nmpool,
            tc.tile_pool(name="tmp", bufs=8) as tmp,
            tc.tile_pool(name="opool", bufs=8) as opool,
            tc.tile_pool(name="psum", bufs=8, space="PSUM") as psum,
        ):
            # ── x cache fill ────────────────────────────────────────────
            # xnm (needed first, by phase A) rides the ACT HWDGE queue,
            # which is otherwise idle early; odd-kp xnn go on GpSimd;
            # even-kp xnn DMAs are emitted lazily between W issues on Sync
            # so they never head-of-line-block the W stream.
            xm = []
            for kt in range(KNM_TILES):
                t = xpool.tile([128, BLOC], bf16, tag=f"xm{kt}")
                nc.scalar.dma_start(t[:], xnm[kt * 128 : (kt + 1) * 128, :])
                xm.append(t)
            xn = []
            x_pending = []
            for kp in range(KP):
                t = xpool.tile([128, 2, BLOC], f8, tag=f"xn{kp}")
                src = xnn[kp * 128 : (kp + 1) * 128, :].rearrange(
                    "p (i b) -> p i b", i=2
                )
                if kp % 2 == 1:
                    nc.gpsimd.dma_start(t[:], src)
                else:
                    x_pending.append((t, src))
                xn.append(t)
            x_pending.reverse()  # pop() from the front of the schedule

            def feed_x(n):
                for _ in range(n):
                    if x_pending:
                        t, src = x_pending.pop()
                        nc.sync.dma_start(t[:], src)

            def osl(ot):
                return slice(ot * 128, (ot + 1) * 128)

            def bsl(bh):
                return slice(bh * 512, (bh + 1) * 512)

            def nmda_phase(ot):
                wg = wmpool.tile([128, KNM_TILES, 128], bf16, tag="wm", name=f"wm_{ot}")
                nc.sync.dma_start(
                    wg[:], wnm[osl(ot), :].rearrange("p (k o) -> p k o", k=KNM_TILES)
                )
                feed_x(1)
                psn = [
                    psum.tile([128, 512], f32, tag="ps", name=f"psn_{ot}_{i}")
                    for i in range(NBH)
                ]
                for kt in range(KNM_TILES):
                    for bh in range(NBH):
                        nc.tensor.matmul(
                            psn[bh][:],
                            lhsT=wg[:, kt, :],
                            rhs=xm[kt][:, bsl(bh)],
                            start=(kt == 0),
                            stop=(kt == KNM_TILES - 1),
                        )
                nm = []
                for bh in range(NBH):
                    t = nmpool.tile([128, 512], f32, tag="nm", name=f"nm_{ot}_{bh}")
                    nc.scalar.copy(t[:], psn[bh][:])
                    nm.append(t)
                return nm

            wn_tiles = {}

            def prefetch_wn(ot):
                if ot not in wn_tiles:
                    wg = wnpool.tile([128, KP, 2, 128], f8, tag="wn", name=f"wn_{ot}")
                    nc.sync.dma_start(
                        wg[:],
                        wnn[osl(ot), :].rearrange("p (k i o) -> p k i o", k=KP, i=2),
                    )
                    feed_x(1)
                    wn_tiles[ot] = wg

            def get_wn(ot):
                prefetch_wn(ot)
                return wn_tiles.pop(ot)

            def non_phase(ot):
                wg = get_wn(ot)
                ps = [
                    psum.tile([128, 512], f32, tag="ps", name=f"ps_{ot}_{i}")
                    for i in range(NBH)
                ]
                for kp in range(KP):
                    for bh in range(NBH):
                        nc.tensor.matmul(
                            ps[bh][:],
                            lhsT=wg[:, kp, :, :],
                            rhs=xn[kp][:, :, bsl(bh)],
                            start=(kp == 0),
                            stop=(kp == KP - 1),
                            perf_mode=DR,
                        )
                return ps

            def epilogue_pair(ot, ps_pair, nm_pair):
                # psum = S_W*(z+b); pre = nm - sigmoid(-(z+b));
                # out = pre^2/(KD+pre^2) = 1 - KD/(KD+pre^2).  Chains
                # interleaved so ACT and DVE overlap across the batch halves.
                sig = [
                    tmp.tile([128, 512], f32, tag="t", name=f"sig_{ot}_{bh}")
                    for bh in range(NBH)
                ]
                rec = [
                    tmp.tile([128, 512], f32, tag="t", name=f"rec_{ot}_{bh}")
                    for bh in range(NBH)
                ]
                ob = [
                    opool.tile([128, 512], bf16, tag="o", name=f"ob_{ot}_{bh}")
                    for bh in range(NBH)
                ]
                for bh in range(NBH):
                    nc.scalar.activation(
                        sig[bh][:], ps_pair[bh][:], ACT.Sigmoid, scale=-1.0 / S_W
                    )
                for bh in range(NBH):
                    nc.vector.tensor_sub(sig[bh][:], nm_pair[bh][:], sig[bh][:])  # pre
                for bh in range(NBH):
                    nc.scalar.activation(nm_pair[bh][:], sig[bh][:], ACT.Square)
                for bh in range(NBH):
                    nc.vector.tensor_scalar_add(sig[bh][:], nm_pair[bh][:], KD)
                for bh in range(NBH):
                    nc.vector.reciprocal_approx_fast(rec[bh][:], sig[bh][:])
                for bh in range(NBH):
                    nc.vector.tensor_scalar(
                        ob[bh][:], rec[bh][:], -KD, 1.0,
                        mybir.AluOpType.mult, mybir.AluOpType.add,
                    )
                for bh in range(NBH):
                    # stores go on the GpSimd SWDGE queue, idle after the x
                    # fill: a store trigger blocked on data-readiness on the
                    # ACT queue would head-of-line-block the epilogue stream
                    # (costs ~25 us of end-of-kernel tail + psum-WAR stalls).
                    nc.gpsimd.dma_start(outT[osl(ot), bsl(bh)], ob[bh][:])

            # ── Phase A: nmda for the first OT_AHEAD o-tiles (xnm only) ──
            nm_ahead = []
            for ot in range(OT_AHEAD):
                nm_ahead.append(nmda_phase(ot))
                if ot == 2:
                    prefetch_wn(0)
                elif ot == 5:
                    prefetch_wn(1)

            # ── Phase B: non + epilogue for the ahead o-tiles ──
            for ot in range(OT_AHEAD):
                ps = non_phase(ot)
                epilogue_pair(ot, ps, nm_ahead[ot])

            # ── Phase C: remaining o-tiles, plain o-outer loop ──
            for ot in range(OT_AHEAD, OT):
                nm = nmda_phase(ot)
                ps = non_phase(ot)
                epilogue_pair(ot, ps, nm)
    nc.compile()
    return nc


def _warmup():
    """Tiny throwaway NEFF run: the first execution after session start
    occasionally dies with NRT_EXEC_UNIT_UNRECOVERABLE; absorb that here."""
    import concourse.bacc as bacc
    import concourse.tile as tile
    import concourse.mybir as mybir
    from concourse.bass_utils import run_bass_kernel_spmd

    nc = bacc.Bacc(None, target_bir_lowering=False)
    a = nc.dram_tensor("a", [128, 128], mybir.dt.float32, kind="ExternalInput")
    b = nc.dram_tensor("b", [128, 128], mybir.dt.float32, kind="ExternalOutput")
    with tile.TileContext(nc) as tc:
        with tc.tile_pool(name="p", bufs=1) as pool:
            t = pool.tile([128, 128], mybir.dt.float32)
            nc.sync.dma_start(t[:], a[:])
            nc.sync.dma_start(b[:], t[:])
    nc.compile()
    ins = [{"a": np.zeros((128, 128), np.float32)} for _ in range(NCORES)]
    for _ in range(3):
        try:
            run_bass_kernel_spmd(nc, ins, core_ids=list(range(NCORES)))
            return
        except Exception:
            continue


def kernel(x, W_nmda, W_non, b_non):
    from concourse.bass_utils import run_bass_kernel_spmd

    x = np.asarray(x, dtype=np.float32)
    W_nmda = np.asarray(W_nmda, dtype=np.float32)
    W_non = np.asarray(W_non, dtype=np.float32)
    b_non = np.asarray(b_non, dtype=np.float32)

    coeff = np.full((IC,), 2.0, dtype=np.float32)
    coeff[0] = 1.0
    coeff[-1] = 1.0

    bf16 = ml_dtypes.bfloat16
    f8 = ml_dtypes.float8_e4m3

    # x, nmda part: [512, B] bf16
    xTm = np.zeros((KNM_PAD, B), dtype=np.float32)
    xTm[0:IC] = x[:, :IC].T
    xTm = xTm.astype(bf16)

    # x, non part: logical k = kp*256 + i*128 + p -> [15*128 rows, 2*B] fp8
    xTn = np.zeros((KNN_PAD, B), dtype=np.float32)
    xTn[0:INC] = x[:, IC:].T
    xTn[INC] = 1.0  # bias row
    xTn = (
        xTn.reshape(KP, 2, 128, B).transpose(0, 2, 1, 3).reshape(KP * 128, 2 * B)
    ).astype(f8)

    # W, nmda part: row ot*128+p, col kt*128+o, bf16
    wTm = np.zeros((KNM_PAD, OUT_F), dtype=np.float32)
    wTm[0:IC] = (W_nmda * coeff[None, :]).T
    wnm = (
        wTm.reshape(KNM_TILES, 128, OT, 128)
        .transpose(2, 1, 0, 3)
        .reshape(OUT_F, KNM_PAD)
    ).astype(bf16)

    # W, non part (pre-scaled by S_W): row ot*128+p, col kp*256+i*128+o, fp8
    wTn = np.zeros((KNN_PAD, OUT_F), dtype=np.float32)
    wTn[0:INC] = W_non.T * S_W
    wTn[INC] = b_non * S_W
    wnn = (
        wTn.reshape(KP, 2, 128, OT, 128)
        .transpose(3, 2, 0, 1, 4)
        .reshape(OUT_F, KP * 256)
    ).astype(f8)

    in_maps = [
        {
            "xnm": np.ascontiguousarray(
                xTm.reshape(KNM_PAD, NCORES, BLOC)[:, c, :]
            ),
            "xnn": np.ascontiguousarray(
                xTn.reshape(KP * 128, 2, NCORES, BLOC)[:, :, c, :].reshape(
                    KP * 128, 2 * BLOC
                )
            ),
            "wnm": wnm,
            "wnn": wnn,
        }
        for c in range(NCORES)
    ]

    if not _nc_cache:
        _warmup()
        _nc_cache.append(_build())
    nc = _nc_cache[0]

    res = None
    last_exc = None
    for _attempt in range(3):
        try:
            res = run_bass_kernel_spmd(nc, in_maps, core_ids=list(range(NCORES)))
            break
        except Exception as e:  # transient device errors (e.g. first-run NRT hiccup)
            last_exc = e
    if res is None:
        raise last_exc

    global LAST_RESULT
    LAST_RESULT = res

    out = np.empty((B, OUT_F), dtype=np.float32)
    for c in range(NCORES):
        out[c * BLOC : (c + 1) * BLOC] = res.results[c]["outT"].astype(np.float32).T
    return out


LAST_RESULT = None


# revision 15
# speedup vs baseline: 1.8439x; 1.0293x over previous
"""DendriticFullyConnected Trainium2 kernel — mixed bf16 / fp8-DoubleRow.

Math (per reference):
  x_c  = x[:, :409];  x_nc = x[:, 409:]
  state = sigmoid(x_nc @ W_non.T + b_non) - 1
  cluster = (x_c * coeff) @ W_nmda.T          # coeff = [1,2,...,2,1]
  pre = cluster + state
  out = pre^2 / (0.25 + pre^2)

Strategy: data-parallel over batch on 8 cores (1024 rows each), weights
replicated.  The contraction splits by precision sensitivity:

  nmda part (K=409->512, 4 k-tiles)  : bf16.  cluster hits the Hill directly
    (sigma~2, gain ~1), so fp8 here costs ~5e-2 rel err.  bf16 keeps it at
    ~3e-3 and runs at the same 1 cycle/row as fp32r, with half the DMA.
  non part (K=3687+bias->3840, 15 pairs of k-tiles): fp8 e4m3 with
    perf_mode=DoubleRow (2 fp8 weights per PE cell -> 256-deep contraction
    per matmul at ~0.55 cycle/row).  The sigmoid's <=0.25 gain squashes the
    fp8 quantization noise (measured 6.3e-3 rel-l2 end to end, vs 2e-2 gate).
    W_non/b_non are pre-scaled by 64 so sigma~1 lands mid e4m3 range (away
    from subnormals); the 1/64 is folded into the sigmoid activation scale.

Layouts are all host-prepared so every DMA is a straight contiguous copy:
  xnm [512, 1024/core] bf16;  xnn [15kp*128p, 2i*1024b] fp8 (i = DoubleRow
  half, logical k = kp*256 + i*128 + p);  wnm rows ot*128+p, cols kt*128+o;
  wnn rows ot*128+p, cols kp*256 + i*128 + o.  Bias rides as x-row 3687
  (ones) paired with b_non*64 in wnn.

Device: outT[o, b] = sum_k wt[k, o] xt[k, b] with W-stationary matmuls
(lhsT = w tile, rhs = cached x), two PSUM groups (nmda / non) per o-tile,
then sigmoid + Hill epilogue on ACT/DVE.  Host transposes back.

Scheduling: the x shard (~5 MB) is cached in SBUF.  Phase A runs the bf16
nmda phases of the first OT_AHEAD o-tiles (they only need the 1 MB xnm)
while the xnn fill streams (xnm + odd kp on GpSimd/SWDGE, even kp lazily
interleaved with the W stream on Sync/HWDGE).  Phases B/C are the plain
o-outer loop; W (622 KB/o-tile) streams on Sync, output stores ride the
ACT HWDGE queue.
"""

import numpy as np
import ml_dtypes

B = 8192
IN_F = 4096
OUT_F = 4096
IC = 409                      # clustering synapses
INC = IN_F - IC               # 3687
KD = 0.25                     # Hill k_d = k_a^n = 0.5^2
NCORES = 8
BLOC = B // NCORES            # 1024
OT = OUT_F // 128             # 32 output-row tiles
NBH = BLOC // 512             # 2 batch halves (512 = max fp32 matmul free dim)
OT_AHEAD = 10                 # o-tiles whose nmda phase covers the x fill

KNM_PAD = 512                 # nmda contraction, padded (4 k-tiles, bf16)
KNM_TILES = 4
KNN = INC + 1                 # 3688: non contraction + bias row
KP = 15                       # fp8 DoubleRow k-pairs (15 * 256 = 3840)
KNN_PAD = KP * 256
S_W = 64.0                    # fp8 pre-scale on W_non/b_non

_nc_cache = []


def _build():
    import concourse.bacc as bacc
    import concourse.tile as tile
    import concourse.mybir as mybir

    f32 = mybir.dt.float32
    bf16 = mybir.dt.bfloat16
    f8 = mybir.dt.float8e4
    ACT = mybir.ActivationFunctionType
    DR = mybir.MatmulPerfMode.DoubleRow

    nc = bacc.Bacc(None, target_bir_lowering=False)
    xnm = nc.dram_tensor("xnm", [KNM_PAD, BLOC], bf16, kind="ExternalInput")
    xnn = nc.dram_tensor("xnn", [KP * 128, 2 * BLOC], f8, kind="ExternalInput")
    wnm = nc.dram_tensor("wnm", [OUT_F, KNM_PAD], bf16, kind="ExternalInput")
    wnn = nc.dram_tensor("wnn", [OUT_F, KP * 256], f8, kind="ExternalInput")
    outT = nc.dram_tensor("outT", [OUT_F, BLOC], bf16, kind="ExternalOutput")

    with tile.TileContext(nc) as tc:
        with (
            tc.tile_pool(name="xpool", bufs=1) as xpool,
            tc.tile_pool(name="wmpool", bufs=6) as wmpool,
            tc.tile_pool(name="wnpool", bufs=4) as wnpool,
            tc.tile_pool(name="nmpool", bufs=24) as nmpool,
            tc.tile_pool(name="tmp", bufs=8) as tmp,
            tc.tile_pool(name="opool", bufs=8) as opool,
            tc.tile_pool(name="psum", bufs=8, space="PSUM") as psum,
        ):
            # ── x cache fill ────────────────────────────────────────────
            # xnm (needed first, by phase A) rides the ACT HWDGE queue,
            # which is otherwise idle early; odd-kp xnn go on GpSimd;
            # even-kp xnn DMAs are emitted lazily between W issues on Sync
            # so they never head-of-line-block the W stream.
            xm = []
            for kt in range(KNM_TILES):
                t = xpool.tile([128, BLOC], bf16, tag=f"xm{kt}")
                src = xnm[kt * 128 : (kt + 1) * 128, :]
                # alternate queues so xm[kt] arrivals pace the k-outer sweep
                if kt % 2 == 0:
                    nc.scalar.dma_start(t[:], src)
                else:
                    nc.gpsimd.dma_start(t[:], src)
                xm.append(t)
            xn = []
            x_pending = []
            for kp in range(KP):
                t = xpool.tile([128, 2, BLOC], f8, tag=f"xn{kp}")
                src = xnn[kp * 128 : (kp + 1) * 128, :].rearrange(
                    "p (i b) -> p i b", i=2
                )
                if kp % 2 == 1:
                    nc.gpsimd.dma_start(t[:], src)
                else:
                    x_pending.append((t, src))
                xn.append(t)
            x_pending.reverse()  # pop() from the front of the schedule

            def feed_x(n):
                for _ in range(n):
                    if x_pending:
                        t, src = x_pending.pop()
                        nc.sync.dma_start(t[:], src)

            def osl(ot):
                return slice(ot * 128, (ot + 1) * 128)

            def bsl(bh):
                return slice(bh * 512, (bh + 1) * 512)

            def load_wm(ot):
                wg = wmpool.tile([128, KNM_TILES, 128], bf16, tag="wm", name=f"wm_{ot}")
                nc.sync.dma_start(
                    wg[:], wnm[osl(ot), :].rearrange("p (k o) -> p k o", k=KNM_TILES)
                )
                feed_x(1)
                return wg

            def nmda_group(ots):
                # k-OUTER over a group of o-tiles (<=4: psum budget): during
                # the x fill each arriving xm[kt] unlocks len(ots)*2 matmuls
                # instead of 2, keeping the PE fed while xnm streams in.
                wgs = [load_wm(ot) for ot in ots]
                psn = [
                    [
                        psum.tile([128, 512], f32, tag="ps", name=f"psn_{ot}_{i}")
                        for i in range(NBH)
                    ]
                    for ot in ots
                ]
                for kt in range(KNM_TILES):
                    for j in range(len(ots)):
                        for bh in range(NBH):
                            nc.tensor.matmul(
                                psn[j][bh][:],
                                lhsT=wgs[j][:, kt, :],
                                rhs=xm[kt][:, bsl(bh)],
                                start=(kt == 0),
                                stop=(kt == KNM_TILES - 1),
                            )
                nms = []
                for j, ot in enumerate(ots):
                    nm = []
                    for bh in range(NBH):
                        t = nmpool.tile([128, 512], f32, tag="nm", name=f"nm_{ot}_{bh}")
                        nc.scalar.copy(t[:], psn[j][bh][:])
                        nm.append(t)
                    nms.append(nm)
                return nms

            def nmda_phase(ot):
                return nmda_group([ot])[0]

            wn_tiles = {}

            def prefetch_wn(ot):
                if ot not in wn_tiles:
                    wg = wnpool.tile([128, KP, 2, 128], f8, tag="wn", name=f"wn_{ot}")
                    nc.sync.dma_start(
                        wg[:],
                        wnn[osl(ot), :].rearrange("p (k i o) -> p k i o", k=KP, i=2),
                    )
                    feed_x(1)
                    wn_tiles[ot] = wg

            def get_wn(ot):
                prefetch_wn(ot)
                return wn_tiles.pop(ot)

            def non_phase(ot):
                wg = get_wn(ot)
                ps = [
                    psum.tile([128, 512], f32, tag="ps", name=f"ps_{ot}_{i}")
                    for i in range(NBH)
                ]
                for kp in range(KP):
                    for bh in range(NBH):
                        nc.tensor.matmul(
                            ps[bh][:],
                            lhsT=wg[:, kp, :, :],
                            rhs=xn[kp][:, :, bsl(bh)],
                            start=(kp == 0),
                            stop=(kp == KP - 1),
                            perf_mode=DR,
                        )
                return ps

            def epilogue_pair(ot, ps_pair, nm_pair):
                # psum = S_W*(z+b); pre = nm - sigmoid(-(z+b));
                # out = pre^2/(KD+pre^2) = 1 - KD/(KD+pre^2).  Chains
                # interleaved so ACT and DVE overlap across the batch halves.
                sig = [
                    tmp.tile([128, 512], f32, tag="t", name=f"sig_{ot}_{bh}")
                    for bh in range(NBH)
                ]
                rec = [
                    tmp.tile([128, 512], f32, tag="t", name=f"rec_{ot}_{bh}")
                    for bh in range(NBH)
                ]
                ob = [
                    opool.tile([128, 512], bf16, tag="o", name=f"ob_{ot}_{bh}")
                    for bh in range(NBH)
                ]
                for bh in range(NBH):
                    nc.scalar.activation(
                        sig[bh][:], ps_pair[bh][:], ACT.Sigmoid, scale=-1.0 / S_W
                    )
                for bh in range(NBH):
                    nc.vector.tensor_sub(sig[bh][:], nm_pair[bh][:], sig[bh][:])  # pre
                for bh in range(NBH):
                    nc.scalar.activation(nm_pair[bh][:], sig[bh][:], ACT.Square)
                for bh in range(NBH):
                    nc.vector.tensor_scalar_add(sig[bh][:], nm_pair[bh][:], KD)
                for bh in range(NBH):
                    nc.vector.reciprocal_approx_fast(rec[bh][:], sig[bh][:])
                for bh in range(NBH):
                    nc.vector.tensor_scalar(
                        ob[bh][:], rec[bh][:], -KD, 1.0,
                        mybir.AluOpType.mult, mybir.AluOpType.add,
                    )
                for bh in range(NBH):
                    # stores go on the GpSimd SWDGE queue, idle after the x
                    # fill: a store trigger blocked on data-readiness on the
                    # ACT queue would head-of-line-block the epilogue stream
                    # (costs ~25 us of end-of-kernel tail + psum-WAR stalls).
                    # The last o-tiles' stores go back on ACT/HWDGE: their
                    # data is ready when triggered (no HoL risk), and the
                    # final SWDGE drain (~7 us completion lag) leaves the
                    # teardown's critical path.
                    if ot >= OT - 2:
                        nc.scalar.dma_start(outT[osl(ot), bsl(bh)], ob[bh][:])
                    else:
                        nc.gpsimd.dma_start(outT[osl(ot), bsl(bh)], ob[bh][:])

            # ── Phase A: nmda for the first OT_AHEAD o-tiles (xnm only) ──
            nm_ahead = []
            groups = [
                list(range(g, min(g + 4, OT_AHEAD))) for g in range(0, OT_AHEAD, 4)
            ]
            for gi, grp in enumerate(groups):
                nm_ahead.extend(nmda_group(grp))
                if gi < 2:
                    prefetch_wn(gi)

            # ── Phase B: non + epilogue for the ahead o-tiles ──
            for ot in range(OT_AHEAD):
                ps = non_phase(ot)
                epilogue_pair(ot, ps, nm_ahead[ot])

            # ── Phase C: remaining o-tiles, plain o-outer loop ──
            for ot in range(OT_AHEAD, OT):
                nm = nmda_phase(ot)
                ps = non_phase(ot)
                epilogue_pair(ot, ps, nm)
    nc.compile()
    return nc


def _warmup():
    """Tiny throwaway NEFF run: the first execution after session start
    occasionally dies with NRT_EXEC_UNIT_UNRECOVERABLE; absorb that here."""
    import concourse.bacc as bacc
    import concourse.tile as tile
    import concourse.mybir as mybir
    from concourse.bass_utils import run_bass_kernel_spmd

    nc = bacc.Bacc(None, target_bir_lowering=False)
    a = nc.dram_tensor("a", [128, 128], mybir.dt.float32, kind="ExternalInput")
    b = nc.dram_tensor("b", [128, 128], mybir.dt.float32, kind="ExternalOutput")
    with tile.TileContext(nc) as tc:
        with tc.tile_pool(name="p", bufs=1) as pool:
            t = pool.tile([128, 128], mybir.dt.float32)
            nc.sync.dma_start(t[:], a[:])
            nc.sync.dma_start(b[:], t[:])
    nc.compile()
    ins = [{"a": np.zeros((128, 128), np.float32)} for _ in range(NCORES)]
    for _ in range(3):
        try:
            run_bass_kernel_spmd(nc, ins, core_ids=list(range(NCORES)))
            return
        except Exception:
            continue


def kernel(x, W_nmda, W_non, b_non):
    from concourse.bass_utils import run_bass_kernel_spmd

    x = np.asarray(x, dtype=np.float32)
    W_nmda = np.asarray(W_nmda, dtype=np.float32)
    W_non = np.asarray(W_non, dtype=np.float32)
    b_non = np.asarray(b_non, dtype=np.float32)

    coeff = np.full((IC,), 2.0, dtype=np.float32)
    coeff[0] = 1.0
    coeff[-1] = 1.0

    bf16 = ml_dtypes.bfloat16
    f8 = ml_dtypes.float8_e4m3

    # x, nmda part: [512, B] bf16
    xTm = np.zeros((KNM_PAD, B), dtype=np.float32)
    xTm[0:IC] = x[:, :IC].T
    xTm = xTm.astype(bf16)

    # x, non part: logical k = kp*256 + i*128 + p -> [15*128 rows, 2*B] fp8
    xTn = np.zeros((KNN_PAD, B), dtype=np.float32)
    xTn[0:INC] = x[:, IC:].T
    xTn[INC] = 1.0  # bias row
    xTn = (
        xTn.reshape(KP, 2, 128, B).transpose(0, 2, 1, 3).reshape(KP * 128, 2 * B)
    ).astype(f8)

    # W, nmda part: row ot*128+p, col kt*128+o, bf16
    wTm = np.zeros((KNM_PAD, OUT_F), dtype=np.float32)
    wTm[0:IC] = (W_nmda * coeff[None, :]).T
    wnm = (
        wTm.reshape(KNM_TILES, 128, OT, 128)
        .transpose(2, 1, 0, 3)
        .reshape(OUT_F, KNM_PAD)
    ).astype(bf16)

    # W, non part (pre-scaled by S_W): row ot*128+p, col kp*256+i*128+o, fp8
    wTn = np.zeros((KNN_PAD, OUT_F), dtype=np.float32)
    wTn[0:INC] = W_non.T * S_W
    wTn[INC] = b_non * S_W
    wnn = (
        wTn.reshape(KP, 2, 128, OT, 128)
        .transpose(3, 2, 0, 1, 4)
        .reshape(OUT_F, KP * 256)
    ).astype(f8)

    in_maps = [
        {
            "xnm": np.ascontiguousarray(
                xTm.reshape(KNM_PAD, NCORES, BLOC)[:, c, :]
            ),
            "xnn": np.ascontiguousarray(
                xTn.reshape(KP * 128, 2, NCORES, BLOC)[:, :, c, :].reshape(
                    KP * 128, 2 * BLOC
                )
            ),
            "wnm": wnm,
            "wnn": wnn,
        }
        for c in range(NCORES)
    ]

    if not _nc_cache:
        _warmup()
        _nc_cache.append(_build())
    nc = _nc_cache[0]

    res = None
    last_exc = None
    for _attempt in range(3):
        try:
            res = run_bass_kernel_spmd(nc, in_maps, core_ids=list(range(NCORES)))
            break
        except Exception as e:  # transient device errors (e.g. first-run NRT hiccup)
            last_exc = e
    if res is None:
        raise last_exc

    global LAST_RESULT
    LAST_RESULT = res

    out = np.empty((B, OUT_F), dtype=np.float32)
    for c in range(NCORES):
        out[c * BLOC : (c + 1) * BLOC] = res.results[c]["outT"].astype(np.float32).T
    return out


LAST_RESULT = None
